# revision 5
# baseline (speedup 1.0000x reference)
"""Causal GQA cross-attention kernel for Trainium2, 8-core SPMD.

Problem: q [2, 2048, 16, 128] f32, kv [2, 2048, 2, 8, 128] f32 ->
out [2, 2048, 16, 128] f32; causal mask (Sq == Sk), GQA with 2 q heads
per kv head, softmax scale 1/sqrt(128).

Sharding: 2 batches x 4 kv-head-pairs -> 8 cores. Each core gets 4 q
heads + 2 kv heads (its GQA groups), computes attention locally; no
collectives. Host splits/gathers.

Per-core algorithm (per q head):
  - Transpose Q and K to d-major layout [128d, 2048s] via PE transposes
    (f32r, 1.5 cyc/row).
  - Scores transposed: S^T[k, q] = (K^T block).T @ Q^T  (f32r matmul,
    moving free dim 512 -> full PE rate).
  - P^T = exp(S^T * scale) on ACT engine, output cast to bf16;
    multiplicative causal mask (bf16 0/1 tile) on diagonal blocks.
  - PV: out[q, d|denom] += (P^T block).T @ [V | ones]  (bf16 matmul,
    PSUM accumulate over k blocks). The ones column yields the softmax
    denominator for free.
  - Store unnormalized [q, 129] to DRAM; host divides by col 128.
Causal block skipping: only k blocks <= q block are computed (~53%).
"""

import math
import os
import sys

import numpy as np

sys.path.insert(0, "/opt/trn_rl_repo")

import concourse.bass as bass  # noqa: E402
import concourse.mybir as mybir  # noqa: E402
import concourse.tile as tile  # noqa: E402
from concourse import bacc  # noqa: E402
from concourse.bass_utils import run_bass_kernel_spmd  # noqa: E402
from concourse.masks import make_identity  # noqa: E402

B, SQ, SK, H, HKV, D = 2, 2048, 2048, 16, 8, 128
NCORES = 8
NQH = H * B // NCORES  # 4 q heads per core
NKVH = HKV * B // NCORES  # 2 kv heads per core
P = 128
NQB = SQ // P  # 16 q blocks of 128
NSB = 4  # q superblocks of 512
SBW = 512
NKB = SK // P  # 16 k blocks
SCALE = 1.0 / math.sqrt(D)

F32 = mybir.dt.float32
F32R = mybir.dt.float32r
BF16 = mybir.dt.bfloat16

LAST_RESULTS = None
_CACHE = {}


def build_module():
    nc = bacc.Bacc(None, target_bir_lowering=False)

    q_d = nc.dram_tensor("q", [NQH, SQ, D], F32, kind="ExternalInput")
    k_d = nc.dram_tensor("k", [NKVH, SK, D], F32, kind="ExternalInput")
    v_d = nc.dram_tensor("v", [NKVH, SK, D], F32, kind="ExternalInput")
    o_d = nc.dram_tensor("o", [NQH, NQB, P, D + 1], F32, kind="ExternalOutput")

    with tile.TileContext(nc) as tc:
        with (
            tc.tile_pool(name="const", bufs=1) as constp,
            tc.tile_pool(name="kt", bufs=2) as ktp,
            tc.tile_pool(name="qt", bufs=2) as qtp,
            tc.tile_pool(name="vaug", bufs=2) as vap,
            tc.tile_pool(name="stage", bufs=4) as stagep,
            tc.tile_pool(name="pt", bufs=4) as ptp,
            tc.tile_pool(name="outs", bufs=4) as outp,
            tc.tile_pool(name="pst", bufs=3, space="PSUM") as pstp,
            tc.tile_pool(name="ppv", bufs=4, space="PSUM") as ppvp,
        ):
            identity = constp.tile([P, P], F32, tag="identity")
            make_identity(nc, identity[:])

            # masks[r][k, q] = 1.0 where (q - k - 128 r) >= 0 else 0.0
            masks = []
            for r in range(4):
                m = constp.tile([P, SBW], BF16, tag=f"mask{r}")
                nc.gpsimd.memset(m[:], 1.0)
                nc.gpsimd.affine_select(
                    out=m[:],
                    in_=m[:],
                    compare_op=mybir.AluOpType.is_ge,
                    fill=0.0,
                    base=-P * r,
                    pattern=[[1, SBW]],
                    channel_multiplier=-1,
                )
                masks.append(m)

            def build_T(dst, src_dram):
                # dst: SBUF [128, 2048] f32r (d-major); src: DRAM [2048, 128]
                for sb in range(NSB):
                    pst = pstp.tile([P, SBW], F32, tag="pst")
                    for i in range(4):
                        blk = sb * 4 + i
                        stg = stagep.tile([P, D], F32, tag="stage")
                        nc.sync.dma_start(
                            stg[:], src_dram[blk * P : (blk + 1) * P, :]
                        )
                        nc.tensor.transpose(
                            pst[:, i * P : (i + 1) * P], stg[:], identity[:]
                        )
                    nc.vector.tensor_copy(
                        dst[:, sb * SBW : (sb + 1) * SBW].bitcast(F32R), pst[:]
                    )

            def head_compute(h, qt, kt_g, vaug_g):
                for sb in range(NSB):
                    pvs = [
                        ppvp.tile([P, D + 1], F32, tag="ppv", name=f"pv_{h}_{sb}_{j}")
                        for j in range(4)
                    ]
                    for kb in range(4 * sb + 4):
                        st = pstp.tile([P, SBW], F32, tag="pst")
                        nc.tensor.matmul(
                            st[:],
                            kt_g[:, kb * P : (kb + 1) * P].bitcast(F32R),
                            qt[:, sb * SBW : (sb + 1) * SBW].bitcast(F32R),
                            start=True,
                            stop=True,
                        )
                        pt = ptp.tile([P, SBW], BF16, tag="pt")
                        nc.scalar.activation(
                            pt[:],
                            st[:],
                            mybir.ActivationFunctionType.Exp,
                            scale=SCALE,
                        )
                        r = kb - 4 * sb
                        if r >= 0:
                            nc.vector.tensor_tensor(
                                out=pt[:],
                                in0=pt[:],
                                in1=masks[r][:],
                                op=mybir.AluOpType.mult,
                            )
                        for j in range(4):
                            qb = 4 * sb + j
                            if kb > qb:
                                continue
                            nc.tensor.matmul(
                                pvs[j][:],
                                pt[:, j * P : (j + 1) * P],
                                vaug_g[:, kb, :],
                                start=(kb == 0),
                                stop=(kb == qb),
                            )
                    for j in range(4):
                        qb = 4 * sb + j
                        ot = outp.tile([P, D + 1], F32, tag="outs")
                        nc.vector.tensor_copy(ot[:], pvs[j][:])
                        nc.sync.dma_start(o_d[h, qb], ot[:])

            for g in range(NKVH):
                kt_g = ktp.tile([P, SK], F32, tag="kt")
                build_T(kt_g[:], k_d[g])
                vaug_g = vap.tile([P, NKB, D + 1], BF16, tag="vaug")
                nc.gpsimd.memset(vaug_g[:, :, D : D + 1], 1.0)
                nc.gpsimd.dma_start(
                    vaug_g[:, :, 0:D],
                    v_d[g].rearrange("(kb p) d -> p kb d", p=P),
                )
                for hl in range(2):
                    h = 2 * g + hl
                    qt = qtp.tile([P, SQ], F32, tag="qt")
                    build_T(qt[:], q_d[h])
                    head_compute(h, qt[:], kt_g[:], vaug_g[:])

    nc.finalize()
    return nc


def _get_module():
    if "nc" not in _CACHE:
        _CACHE["nc"] = build_module()
    return _CACHE["nc"]


def kernel(q, kv):
    global LAST_RESULTS
    q = np.asarray(q, dtype=np.float32)
    kv = np.asarray(kv, dtype=np.float32)

    nc = _get_module()
    in_maps = []
    for c in range(NCORES):
        b, j = divmod(c, 4)
        q_s = np.ascontiguousarray(
            np.transpose(q[b][:, 4 * j : 4 * j + 4, :], (1, 0, 2))
        )
        k_s = np.ascontiguousarray(
            np.transpose(kv[b][:, 0, 2 * j : 2 * j + 2, :], (1, 0, 2))
        )
        v_s = np.ascontiguousarray(
            np.transpose(kv[b][:, 1, 2 * j : 2 * j + 2, :], (1, 0, 2))
        )
        in_maps.append({"q": q_s, "k": k_s, "v": v_s})

    trace = bool(int(os.environ.get("KERNEL_TRACE", "0")))
    kwargs = {}
    tdir = os.environ.get("KERNEL_TRACE_DIR")
    if tdir:
        kwargs["tmpdir"] = tdir
    res = run_bass_kernel_spmd(
        nc, in_maps, core_ids=list(range(NCORES)), trace=trace, **kwargs
    )
    LAST_RESULTS = res

    out = np.empty((B, SQ, H, D), np.float32)
    for c in range(NCORES):
        b, j = divmod(c, 4)
        o = res.results[c]["o"].reshape(NQH, SQ, D + 1)
        norm = o[..., :D] / o[..., D : D + 1]
        out[b, :, 4 * j : 4 * j + 4, :] = np.transpose(norm, (1, 0, 2))
    return out
